# revision 6
# baseline (speedup 1.0000x reference)
"""TSM-style 3-tap depthwise temporal conv on 8 Trainium2 NeuronCores.

out[n, t, c, h, w] = w[c,0]*x[n,t-1,c,h,w] + w[c,1]*x[n,t,c,h,w]
                   + w[c,2]*x[n,t+1,c,h,w]   (zero-padded at clip edges)

Sharding: pure data parallel over the nt (clip-batch) axis — each of the 8
cores gets whole clips (nt=64, n_segment=8 -> one 8-frame clip per core).
Weight (c,3) is replicated.

Per core the kernel streams (frame x channel-block) tiles of shape
(128 partitions, 3136 free) through SBUF:
  ACT:  y  = x[t] * w1          (per-partition scale)
  DVE:  y  = (x[t-1] * w0) + y  (scalar_tensor_tensor, skipped at clip start)
  DVE:  y  = (x[t+1] * w2) + y  (skipped at clip end)
Loads are issued on the SP HWDGE ring, stores on the ACT HWDGE ring so the
two streams don't serialize behind each other. Memory-bound: ~51 MB of HBM
traffic per core.
"""

import numpy as np

import concourse.bacc as bacc
import concourse.mybir as mybir
import concourse.tile as tile
from concourse.bass_utils import run_bass_kernel_spmd

N_CORES = 8
P = 128  # SBUF partitions

_cache = {}


def _emit_conv(nc, tc, pools, src, dst, wt_by_blk, F, C, HW, n_seg, uid):
    """Emit one full conv pass src -> dst (both DRAM (F, C, HW) handles)."""
    wp, xp, yp = pools
    mult = mybir.AluOpType.mult
    add = mybir.AluOpType.add
    nblk = C // P
    for b in range(nblk):
        cs = slice(b * P, (b + 1) * P)
        wt = wt_by_blk[b]
        w0, w1, w2 = wt[:, 0:1], wt[:, 1:2], wt[:, 2:3]

        xt = [None] * F

        def load(t):
            xt[t] = xp.tile([P, HW], mybir.dt.float32, tag="x", name=f"x{uid}_{b}_{t}")
            nc.sync.dma_start(out=xt[t][:], in_=src[t, cs, :])

        def compute_store(t):
            s = t % n_seg
            y = yp.tile([P, HW], mybir.dt.float32, tag="y", name=f"y{uid}_{b}_{t}")
            nc.scalar.mul(y[:], xt[t][:], w1)
            if s > 0:
                nc.vector.scalar_tensor_tensor(y[:], xt[t - 1][:], w0, y[:], mult, add)
            if s < n_seg - 1:
                nc.vector.scalar_tensor_tensor(y[:], xt[t + 1][:], w2, y[:], mult, add)
            nc.scalar.dma_start(out=dst[t, cs, :], in_=y[:])

        load(0)
        if F > 1:
            load(1)
        for t in range(F):
            if t + 2 < F:
                load(t + 2)
            compute_store(t)


def _build(F, C, HW, n_seg, repeat=1, x_bufs=6, y_bufs=4):
    """One-core program: x (F, C, HW) -> out (F, C, HW).

    repeat > 1 chains the conv through internal DRAM ping-pong buffers —
    identical HBM traffic per pass; used by the timing harness.
    """
    nc = bacc.Bacc(
        "TRN2",
        target_bir_lowering=False,
        debug=False,
        num_devices=N_CORES,
    )
    x = nc.dram_tensor("x", (F, C, HW), mybir.dt.float32, kind="ExternalInput")
    w = nc.dram_tensor("weight", (C, 3), mybir.dt.float32, kind="ExternalInput")
    out = nc.dram_tensor("out", (F, C, HW), mybir.dt.float32, kind="ExternalOutput")
    scratch = [
        nc.dram_tensor(f"scratch{i}", (F, C, HW), mybir.dt.float32, kind="Internal")
        for i in range(2 if repeat > 1 else 0)
    ]

    nblk = C // P
    with tile.TileContext(nc) as tc:
        with (
            tc.tile_pool(name="wp", bufs=1) as wp,
            tc.tile_pool(name="xp", bufs=x_bufs) as xp,
            tc.tile_pool(name="yp", bufs=y_bufs) as yp,
        ):
            wt_by_blk = []
            for b in range(nblk):
                wt = wp.tile([P, 3], mybir.dt.float32, tag=f"w{b}", name=f"w{b}")
                nc.sync.dma_start(out=wt[:], in_=w[b * P : (b + 1) * P, :])
                wt_by_blk.append(wt)

            pools = (wp, xp, yp)
            for k in range(repeat):
                src = x if k == 0 else scratch[k % 2]
                dst = out if k == repeat - 1 else scratch[(k + 1) % 2]
                _emit_conv(nc, tc, pools, src, dst, wt_by_blk, F, C, HW, n_seg, k)
    nc.compile()
    return nc


def _get_program(F, C, HW, n_seg, repeat=1):
    key = (F, C, HW, n_seg, repeat)
    if key not in _cache:
        _cache[key] = _build(F, C, HW, n_seg, repeat=repeat)
    return _cache[key]


def kernel(x, weight, n_segment, **_kw):
    x = np.asarray(x)
    weight = np.ascontiguousarray(np.asarray(weight, dtype=np.float32))
    n_seg = int(np.asarray(n_segment))
    nt, C, H, W = x.shape
    HW = H * W
    assert nt % N_CORES == 0
    F = nt // N_CORES
    # each core must hold whole clips
    assert F % n_seg == 0 or n_seg % F == 0, (F, n_seg)

    nc = _get_program(F, C, HW, n_seg)

    xs = np.ascontiguousarray(x, dtype=np.float32).reshape(nt, C, HW)
    in_maps = [
        {"x": xs[i * F : (i + 1) * F], "weight": weight} for i in range(N_CORES)
    ]
    res = run_bass_kernel_spmd(nc, in_maps, list(range(N_CORES)))
    out = np.concatenate([res.results[i]["out"] for i in range(N_CORES)], axis=0)
    return out.reshape(nt, C, H, W).astype(x.dtype, copy=False)


# revision 8
# speedup vs baseline: 4.7018x; 4.7018x over previous
"""TSM-style 3-tap depthwise temporal conv on 8 Trainium2 NeuronCores.

out[n, t, c, h, w] = w[c,0]*x[n,t-1,c,h,w] + w[c,1]*x[n,t,c,h,w]
                   + w[c,2]*x[n,t+1,c,h,w]   (zero-padded at clip edges)

Sharding: pure data parallel over the nt (clip-batch) axis — each of the 8
cores gets whole clips (nt=64, n_segment=8 -> one 8-frame clip per core).
Weight (c,3) is replicated.

This platform has a large fixed cost per *instruction* (measured ~60-100us
on the compute engines, independent of operand size, with DMA transfers
comparatively cheap), so the kernel minimizes instruction count: per
channel-block of 128 channels it loads the whole clip into one SBUF tile
(128 x 8 x 3136), applies the 3-tap conv as three full-clip fused ops on
three different engines, and stores with one DMA:

  ACT:     y          = x * w1                (per-partition scale)
  DVE:     y[:, 1:]  += x[:, :-1] * w0        (scalar_tensor_tensor)
  GPSIMD:  y[:, :-1] += x[:, 1:]  * w2        (scalar_tensor_tensor)

10 instructions per core per pass instead of the naive ~76.
"""

import numpy as np

import concourse.bacc as bacc
import concourse.mybir as mybir
import concourse.tile as tile
from concourse.bass_utils import run_bass_kernel_spmd

N_CORES = 8
P = 128  # SBUF partitions

_cache = {}


def _emit_conv(nc, tc, pools, src, dst, wt_by_blk, F, C, HW, n_seg, uid,
               shift_engine="vector"):
    """Emit one full conv pass src -> dst (both DRAM (F, C, HW) handles)."""
    wp, xp, yp = pools
    mult = mybir.AluOpType.mult
    add = mybir.AluOpType.add
    nblk = C // P
    n_clips = max(F // n_seg, 1)
    S = min(n_seg, F)
    eng2 = getattr(nc, shift_engine)
    for b in range(nblk):
        cs = slice(b * P, (b + 1) * P)
        wt = wt_by_blk[b]
        w0, w1, w2 = wt[:, 0:1], wt[:, 1:2], wt[:, 2:3]

        xt = xp.tile([P, F, HW], mybir.dt.float32, tag="x", name=f"x{uid}_{b}")
        nc.sync.dma_start(out=xt[:], in_=src[:, cs, :].rearrange("f c x -> c f x"))

        y = yp.tile([P, F, HW], mybir.dt.float32, tag="y", name=f"y{uid}_{b}")
        nc.scalar.mul(y[:], xt[:], w1)
        for c in range(n_clips):
            lo, hi = c * S, (c + 1) * S
            nc.vector.scalar_tensor_tensor(
                y[:, lo + 1 : hi, :], xt[:, lo : hi - 1, :], w0,
                y[:, lo + 1 : hi, :], mult, add,
            )
            eng2.scalar_tensor_tensor(
                y[:, lo : hi - 1, :], xt[:, lo + 1 : hi, :], w2,
                y[:, lo : hi - 1, :], mult, add,
            )
        nc.scalar.dma_start(out=dst[:, cs, :].rearrange("f c x -> c f x"), in_=y[:])


def _build(F, C, HW, n_seg, repeat=1, x_bufs=1, y_bufs=1, shift_engine="vector"):
    """One-core program: x (F, C, HW) -> out (F, C, HW).

    repeat > 1 chains the conv through internal DRAM ping-pong buffers —
    identical HBM traffic per pass; used by the timing harness.
    """
    nc = bacc.Bacc(
        "TRN2",
        target_bir_lowering=False,
        debug=False,
        num_devices=N_CORES,
    )
    x = nc.dram_tensor("x", (F, C, HW), mybir.dt.float32, kind="ExternalInput")
    w = nc.dram_tensor("weight", (C, 3), mybir.dt.float32, kind="ExternalInput")
    out = nc.dram_tensor("out", (F, C, HW), mybir.dt.float32, kind="ExternalOutput")
    scratch = [
        nc.dram_tensor(f"scratch{i}", (F, C, HW), mybir.dt.float32, kind="Internal")
        for i in range(2 if repeat > 1 else 0)
    ]

    nblk = C // P
    with tile.TileContext(nc) as tc:
        with (
            tc.tile_pool(name="wp", bufs=1) as wp,
            tc.tile_pool(name="xp", bufs=x_bufs) as xp,
            tc.tile_pool(name="yp", bufs=y_bufs) as yp,
        ):
            wt_by_blk = []
            for b in range(nblk):
                wt = wp.tile([P, 3], mybir.dt.float32, tag=f"w{b}", name=f"w{b}")
                nc.sync.dma_start(out=wt[:], in_=w[b * P : (b + 1) * P, :])
                wt_by_blk.append(wt)

            pools = (wp, xp, yp)
            for k in range(repeat):
                src = x if k == 0 else scratch[k % 2]
                dst = out if k == repeat - 1 else scratch[(k + 1) % 2]
                _emit_conv(nc, tc, pools, src, dst, wt_by_blk, F, C, HW, n_seg, k,
                           shift_engine=shift_engine)
    nc.compile()
    return nc


def _get_program(F, C, HW, n_seg, repeat=1):
    key = (F, C, HW, n_seg, repeat)
    if key not in _cache:
        _cache[key] = _build(F, C, HW, n_seg, repeat=repeat)
    return _cache[key]


def kernel(x, weight, n_segment, **_kw):
    x = np.asarray(x)
    weight = np.ascontiguousarray(np.asarray(weight, dtype=np.float32))
    n_seg = int(np.asarray(n_segment))
    nt, C, H, W = x.shape
    HW = H * W
    assert nt % N_CORES == 0
    F = nt // N_CORES
    # each core must hold whole clips
    assert F % n_seg == 0 or n_seg % F == 0, (F, n_seg)

    nc = _get_program(F, C, HW, n_seg)

    xs = np.ascontiguousarray(x, dtype=np.float32).reshape(nt, C, HW)
    in_maps = [
        {"x": xs[i * F : (i + 1) * F], "weight": weight} for i in range(N_CORES)
    ]
    res = run_bass_kernel_spmd(nc, in_maps, list(range(N_CORES)))
    out = np.concatenate([res.results[i]["out"] for i in range(N_CORES)], axis=0)
    return out.reshape(nt, C, H, W).astype(x.dtype, copy=False)
